# revision 9
# baseline (speedup 1.0000x reference)
"""MHA on 8 NeuronCores, v3: pipelined projections + row-tiled scores.

Core c owns token block c = (batch c//2, seq half c%2), 1024 tokens.
Pair p = heads (2p, 2p+1): K^T/Q^T d-rows 128p..128(p+1) in one tile
(head-even rows 0-63, head-odd rows 64-127).

  - Projections emitted per d-block (K_p, Q_p) / per (token-chunk, half)
    (V), interleaved with attention so ACT's exp stream starts ~10us in.
  - K AllGathers per 128-row block (8 pairwise AGs), V per token quarter
    (4 AGs). All attention K/V inputs are read from AG outputs so the SPMD
    program is core-symmetric (keys in global order).
  - Scores use 64x128 PE row tiling: head-even at tile_position (0,0),
    head-odd at (64,0) -> both heads stream concurrently, recovering the
    2x lost to the 64-deep head_dim contraction.
  - exp on ACT per [128, 2, 512] PSUM group tile (scale=1/8) -> pg bf16.
  - PV per head: 16 x [128k,65]x[128k,512] chained matmuls; 65th V row of
    ones gives the softmax denominator. PV groups are woven between scores
    groups via a queue so the PE alternates and ACT never starves.
  - Normalization via DRAM-bounce reciprocal broadcast, per (pair, qb).
  - wo emitted per query-half so most overlaps the last pair's attention.
"""
import numpy as np
import ml_dtypes
from collections import deque

import concourse.bass as bass
import concourse.bacc as bacc
import concourse.tile as tile
import concourse.mybir as mybir

N_CORES = 8
P = 128
B, S, D = 4, 2048, 1024
TOK = 1024
CD = D // P  # 8 d-blocks
QB = 512
NKC = S // P  # 16 key chunks (global order)
NG = 8  # groups of 2 key chunks
F32 = mybir.dt.float32
BF16 = mybir.dt.bfloat16
EXP = mybir.ActivationFunctionType.Exp
PAIR_GROUPS = [[2 * i, 2 * i + 1] for i in range(4)]

_CACHE = {}


def _n_excess_waits(nc):
    import json

    m = json.loads(nc.to_json_bytes())
    insts = [i for f in m["functions"] for b in f["blocks"] for i in b["instructions"]]
    return sum(
        1
        for i in insts
        if len((i.get("sync_info") or {}).get("on_wait", [])) >= 2
        and i.get("opcode") != "EventSemaphore"
    )


def _finish(nc):
    nc.compile()
    import bass_rust

    for _ in range(6):
        if _n_excess_waits(nc) == 0:
            break
        bass_rust.generate_event_semaphores(nc)
    assert _n_excess_waits(nc) == 0, "excess sync waits remain"
    nc.codegen_inst_isa_subclasses()
    return nc


def build_nc(scopes=False):
    nc = bacc.Bacc("TRN2", target_bir_lowering=False, debug=False, num_devices=N_CORES)

    xqT_d = nc.dram_tensor("xqT", [D, TOK], BF16, kind="ExternalInput").ap()
    xkT_d = nc.dram_tensor("xkT", [D, TOK], BF16, kind="ExternalInput").ap()
    xvT_d = nc.dram_tensor("xvT", [D, TOK], BF16, kind="ExternalInput").ap()
    wqkvT = nc.dram_tensor("wqkvT", [D, 3 * D], BF16, kind="ExternalInput").ap()
    woT = nc.dram_tensor("woT", [D, D], BF16, kind="ExternalInput").ap()
    out = nc.dram_tensor("out", [TOK, D], F32, kind="ExternalOutput").ap()

    kag_i = nc.dram_tensor("kag_i", [D, TOK], BF16).ap()
    kag_os = [nc.dram_tensor(f"kag_o{p}", [2, P, TOK], BF16).ap() for p in range(CD)]
    vag_i = nc.dram_tensor("vag_i", [TOK, D], BF16).ap()
    vag_os = [
        nc.dram_tensor(f"vag_o{q}", [2, TOK // 4, D], BF16).ap() for q in range(4)
    ]
    den_d = nc.dram_tensor("den_d", [16, TOK], F32).ap()
    recip_d = nc.dram_tensor("recip_d", [16, TOK], F32).ap()

    from contextlib import ExitStack, nullcontext

    def scope(name):
        return nc.named_scope(name) if scopes else nullcontext()

    AG_KW = dict(
        kind="AllGather", op=mybir.AluOpType.bypass, replica_groups=PAIR_GROUPS
    )

    with tile.TileContext(nc) as tc:
        persist = ExitStack()
        ktp = persist.enter_context(tc.tile_pool(name="ktp", bufs=3))
        qtp = persist.enter_context(tc.tile_pool(name="qtp", bufs=3))
        vtp = persist.enter_context(tc.tile_pool(name="vtp", bufs=3))
        ltp = persist.enter_context(tc.tile_pool(name="ltp", bufs=1))
        pgp = persist.enter_context(tc.tile_pool(name="pgp", bufs=7))
        arp = persist.enter_context(tc.tile_pool(name="arp", bufs=2))
        obp = persist.enter_context(tc.tile_pool(name="obp", bufs=1))
        nrm = persist.enter_context(tc.tile_pool(name="nrm", bufs=1))
        s_ps = persist.enter_context(tc.tile_pool(name="s_ps", bufs=2, space="PSUM"))
        pv_ps = persist.enter_context(tc.tile_pool(name="pv_ps", bufs=2, space="PSUM"))
        mm_ps = persist.enter_context(tc.tile_pool(name="mm_ps", bufs=2, space="PSUM"))
        pha = ExitStack()
        xp = pha.enter_context(tc.tile_pool(name="xp", bufs=1))
        wkq = pha.enter_context(tc.tile_pool(name="wkq", bufs=1))
        wvp = pha.enter_context(tc.tile_pool(name="wvp", bufs=1))
        evp = pha.enter_context(tc.tile_pool(name="evp", bufs=3))

        # ---------------- input loads ----------------
        with scope("load"):
            xkT, xqT, xvT = [], [], []
            for nm, src, lst in (("k", xkT_d, xkT), ("q", xqT_d, xqT), ("v", xvT_d, xvT)):
                for j in range(CD):
                    t = xp.tile([P, TOK], BF16, name=f"x{nm}T_{j}")
                    nc.sync.dma_start(out=t, in_=src[j * P : (j + 1) * P, :])
                    lst.append(t)
            # K/Q weight blocks: wk_b[p][r, j, c] = wqkvT[j*128+r, base+128p+c]
            wk_b, wq_b = [], []
            for kind, base, lst in (("k", D, wk_b), ("q", 0, wq_b)):
                for p in range(CD):
                    t = wkq.tile([P, CD, P], BF16, name=f"w{kind}b_{p}")
                    src = bass.AP(
                        tensor=wqkvT.tensor,
                        offset=base + p * P,
                        ap=[[3 * D, P], [P * 3 * D, CD], [1, P]],
                    )
                    nc.sync.dma_start(out=t, in_=src)
                    lst.append(t)
            wv_j = []
            for j in range(CD):
                t = wvp.tile([P, D], BF16, name=f"wv_{j}")
                nc.sync.dma_start(out=t, in_=wqkvT[j * P : (j + 1) * P, 2 * D : 3 * D])
                wv_j.append(t)

        lts = [ltp.tile([P, TOK], BF16, name=f"lt_{i}") for i in range(CD)]

        kT_t = [None] * CD
        qT_t = [None] * CD
        v_t = [None] * CD  # per pair: [128 r, 16 kc, 2 hh, 65]

        # ---------------- emission units ----------------
        def proj_k(p):
            with scope(f"proj_k{p}"):
                for half in range(2):
                    ps = mm_ps.tile([P, QB], F32, name="ps_k", tag="mmps")
                    for j in range(CD):
                        nc.tensor.matmul(
                            ps,
                            wk_b[p][:, j, :],
                            xkT[j][:, half * QB : (half + 1) * QB],
                            start=(j == 0),
                            stop=(j == CD - 1),
                        )
                    sb = evp.tile([P, QB], BF16, name="sb_k", tag="evp")
                    nc.vector.tensor_copy(sb, ps)
                    nc.sync.dma_start(
                        out=kag_i[p * P : (p + 1) * P, half * QB : (half + 1) * QB],
                        in_=sb,
                    )
            with scope(f"ag_k{p}"):
                nc.gpsimd.collective_compute(
                    ins=[kag_i[p * P : (p + 1) * P, :]], outs=[kag_os[p][:]], **AG_KW
                )
            t = ktp.tile([P, S], BF16, name=f"kT_{p}", tag="ktp")
            nc.sync.dma_start(out=t[:, 0:TOK], in_=kag_os[p][0])
            nc.sync.dma_start(out=t[:, TOK:S], in_=kag_os[p][1])
            kT_t[p] = t

        def proj_q(p):
            with scope(f"proj_q{p}"):
                qt = qtp.tile([P, TOK], BF16, name=f"qT_{p}", tag="qtp")
                for half in range(2):
                    ps = mm_ps.tile([P, QB], F32, name="ps_q", tag="mmps")
                    for j in range(CD):
                        nc.tensor.matmul(
                            ps,
                            wq_b[p][:, j, :],
                            xqT[j][:, half * QB : (half + 1) * QB],
                            start=(j == 0),
                            stop=(j == CD - 1),
                        )
                    nc.vector.tensor_copy(qt[:, half * QB : (half + 1) * QB], ps)
                qT_t[p] = qt

        def proj_v(t_i, half):
            with scope(f"proj_v{t_i}_{half}"):
                ps = mm_ps.tile([P, QB], F32, name="ps_v", tag="mmps")
                for j in range(CD):
                    nc.tensor.matmul(
                        ps,
                        xvT[j][:, t_i * P : (t_i + 1) * P],
                        wv_j[j][:, half * QB : (half + 1) * QB],
                        start=(j == 0),
                        stop=(j == CD - 1),
                    )
                sb = evp.tile([P, QB], BF16, name="sb_v", tag="evp")
                nc.vector.tensor_copy(sb, ps)
                nc.sync.dma_start(
                    out=vag_i[t_i * P : (t_i + 1) * P, half * QB : (half + 1) * QB],
                    in_=sb,
                )

        def ag_v(q):
            with scope(f"ag_v{q}"):
                nc.gpsimd.collective_compute(
                    ins=[vag_i[q * TOK // 4 : (q + 1) * TOK // 4, :]],
                    outs=[vag_os[q][:]],
                    **AG_KW,
                )

        def v_fill(p):
            # v_t[p][r, kc, hh, c] = V[key kc*128+r, head dim 64*(2p+hh)+c]
            # global chunk kc = half*8 + 2q + i from vag_os[q][half, i*128+r]
            t = vtp.tile([P, NKC, 2, 65], BF16, name=f"v_{p}", tag="vtp")
            nc.vector.memset(t[:, :, :, 64:65], 1.0)
            for q in range(4):
                for half in range(2):
                    for hh in range(2):
                        src = bass.AP(
                            tensor=vag_os[q].tensor,
                            offset=vag_os[q].offset
                            + (half * (TOK // 4)) * D
                            + 128 * p
                            + 64 * hh,
                            ap=[[D, P], [P * D, 2], [1, 64]],
                        )
                        nc.sync.dma_start(
                            out=t[:, half * 8 + 2 * q : half * 8 + 2 * q + 2, hh, 0:64],
                            in_=src,
                        )
            v_t[p] = t

        # -------- attention --------
        pg_tiles = {}
        pvs_tiles = {}

        def sc_exp(p, qb, g):
            qs = slice(qb * QB, (qb + 1) * QB)
            with scope(f"sc{p}_{qb}_{g}"):
                sg_e = s_ps.tile([P, 2, QB], F32, name="sg_e", tag="s_ps")
                sg_o = s_ps.tile([P, 2, QB], F32, name="sg_o", tag="s_ps")
                for i in range(2):
                    kc = 2 * g + i
                    ks = slice(kc * P, (kc + 1) * P)
                    nc.tensor.matmul(
                        sg_e[:, i, :], kT_t[p][0:64, ks], qT_t[p][0:64, qs],
                        start=True, stop=True, tile_position=(0, 0),
                    )
                    nc.tensor.matmul(
                        sg_o[:, i, :], kT_t[p][64:P, ks], qT_t[p][64:P, qs],
                        start=True, stop=True, tile_position=(64, 0),
                    )
                pg = pgp.tile([P, 2, 2, QB], BF16, name="pg", tag="pgp")
                nc.scalar.activation(pg[:, 0], sg_e, EXP, scale=0.125)
                nc.scalar.activation(pg[:, 1], sg_o, EXP, scale=0.125)
                pg_tiles[(p, qb, g)] = pg

        def pv_part(p, qb, g):
            if g == 0:
                pvs_tiles[(p, qb)] = [
                    pv_ps.tile([65, QB], F32, name=f"pv{hh}", tag="pv_ps")
                    for hh in range(2)
                ]
            pvs = pvs_tiles[(p, qb)]
            pg = pg_tiles.pop((p, qb, g))
            for hh in range(2):
                for i in range(2):
                    kc = 2 * g + i
                    nc.tensor.matmul(
                        pvs[hh],
                        v_t[p][:, kc, hh, :],
                        pg[:, hh, i, :],
                        start=(kc == 0),
                        stop=(kc == NKC - 1),
                    )

        def pv_fin(p, qb):
            with scope(f"fin{p}_{qb}"):
                pvs = pvs_tiles.pop((p, qb))
                ar = arp.tile([65, 2, QB], F32, name="ar", tag="arp")
                for hh in range(2):
                    nc.vector.tensor_copy(ar[:, hh, :], pvs[hh])
                off = 2 * p * TOK + qb * QB
                nc.sync.dma_start(
                    out=bass.AP(tensor=den_d.tensor, offset=off, ap=[[TOK, 2], [1, QB]]),
                    in_=ar[64:65, :, :],
                )
                dsq = nrm.tile([64, 16], F32, name="dsq", tag="dsq")
                nc.sync.dma_start(
                    out=dsq,
                    in_=bass.AP(
                        tensor=den_d.tensor, offset=off, ap=[[8, 64], [1, 8], [TOK, 2]]
                    ),
                )
                rsq = nrm.tile([64, 16], F32, name="rsq", tag="rsq")
                nc.vector.reciprocal(rsq, dsq)
                nc.sync.dma_start(
                    out=bass.AP(
                        tensor=recip_d.tensor, offset=off, ap=[[8, 64], [1, 8], [TOK, 2]]
                    ),
                    in_=rsq,
                )
                bc = nrm.tile([64, 2, QB], F32, name="bc", tag="bc")
                nc.sync.dma_start(
                    out=bc,
                    in_=bass.AP(
                        tensor=recip_d.tensor, offset=off, ap=[[0, 64], [TOK, 2], [1, QB]]
                    ),
                )
                for hh in range(2):
                    rr = slice(64 * hh, 64 * hh + 64)
                    nc.vector.tensor_mul(
                        lts[p][rr, qb * QB : (qb + 1) * QB], ar[0:64, hh, :], bc[:, hh, :]
                    )

        wo_t = [None] * CD

        def load_wo():
            for j in range(CD):
                t = ltp.tile([P, D], BF16, name=f"wo_{j}")
                nc.sync.dma_start(out=t, in_=woT[j * P : (j + 1) * P, :])
                wo_t[j] = t

        def wo_unit(t_i):
            with scope(f"wo{t_i}"):
                ob = obp.tile([P, D], F32, name="ob", tag="ob")
                for hh in range(2):
                    ps3 = mm_ps.tile([P, QB], F32, name="ps3", tag="mmps")
                    for sc_i in range(CD):
                        nc.tensor.matmul(
                            ps3,
                            lts[sc_i][:, t_i * P : (t_i + 1) * P],
                            wo_t[sc_i][:, hh * QB : (hh + 1) * QB],
                            start=(sc_i == 0),
                            stop=(sc_i == CD - 1),
                        )
                    nc.vector.tensor_copy(ob[:, hh * QB : (hh + 1) * QB], ps3)
                nc.sync.dma_start(out=out[t_i * P : (t_i + 1) * P, :], in_=ob)

        # ---------------- master emission sequence ----------------
        # pv work queue: woven between scores groups to keep both PE and ACT fed
        pvq = deque()

        def queue_pv(p, qb):
            for g in range(NG):
                pvq.append(lambda p=p, qb=qb, g=g: pv_part(p, qb, g))
            pvq.append(lambda p=p, qb=qb: pv_fin(p, qb))

        def drain(n):
            for _ in range(min(n, len(pvq))):
                pvq.popleft()()

        DS = [1, 1, 1, 1, 1, 1, 1, 2]  # pv parts drained per scores group

        def unit(p, qb):
            for g in range(NG):
                sc_exp(p, qb, g)
                drain(DS[g])

        proj_k(0); proj_q(0)
        # pair-0 qb0 scores stream while all V projections run on the PE
        pv_units = [(t, h) for h in range(2) for t in range(8)]
        for g in range(NG):
            sc_exp(0, 0, g)
            for t_i, half in pv_units[2 * g : 2 * g + 2]:
                proj_v(t_i, half)
        for q in range(4):
            ag_v(q)
        proj_k(1); proj_q(1)
        v_fill(0); v_fill(1)
        queue_pv(0, 0)
        unit(0, 1)
        queue_pv(0, 1); proj_k(2); proj_q(2); v_fill(2)
        unit(1, 0)
        queue_pv(1, 0); proj_k(3); proj_q(3); v_fill(3)
        unit(1, 1)
        queue_pv(1, 1); load_wo()
        unit(2, 0)
        queue_pv(2, 0); proj_k(4); proj_q(4); v_fill(4)
        unit(2, 1)
        queue_pv(2, 1)
        unit(3, 0)
        queue_pv(3, 0); proj_k(5); proj_q(5); v_fill(5)
        unit(3, 1)
        queue_pv(3, 1)
        unit(4, 0)
        queue_pv(4, 0); proj_k(6); proj_q(6); v_fill(6)
        unit(4, 1)
        queue_pv(4, 1)
        unit(5, 0)
        queue_pv(5, 0); proj_k(7); proj_q(7); v_fill(7)
        unit(5, 1)
        queue_pv(5, 1)
        pha.close()
        unit(6, 0)
        queue_pv(6, 0)
        unit(6, 1)
        queue_pv(6, 1)
        unit(7, 0)
        queue_pv(7, 0)
        unit(7, 1)
        queue_pv(7, 1)
        drain(9)
        wo_unit(0); wo_unit(1)
        drain(9)
        wo_unit(2); wo_unit(3)
        drain(len(pvq))
        for t_i in range(4, 8):
            wo_unit(t_i)

        persist.close()

    return _finish(nc)


def _get_nc(scopes=False):
    key = ("nc", scopes)
    if key not in _CACHE:
        _CACHE[key] = build_nc(scopes)
    return _CACHE[key]


def make_in_maps(query, key, value, wq, wk, wv, wo):
    qf = np.asarray(query, np.float32).reshape(B * S, D)
    kf = np.asarray(key, np.float32).reshape(B * S, D)
    vf = np.asarray(value, np.float32).reshape(B * S, D)
    wqkvT = np.ascontiguousarray(
        np.concatenate([np.asarray(wq), np.asarray(wk), np.asarray(wv)], 0).T
    ).astype(ml_dtypes.bfloat16)
    woT_h = np.ascontiguousarray(np.asarray(wo).T).astype(ml_dtypes.bfloat16)
    in_maps = []
    for c in range(N_CORES):
        sl = slice(c * TOK, (c + 1) * TOK)
        in_maps.append(
            {
                "xqT": np.ascontiguousarray(qf[sl].T).astype(ml_dtypes.bfloat16),
                "xkT": np.ascontiguousarray(kf[sl].T).astype(ml_dtypes.bfloat16),
                "xvT": np.ascontiguousarray(vf[sl].T).astype(ml_dtypes.bfloat16),
                "wqkvT": wqkvT,
                "woT": woT_h,
            }
        )
    return in_maps


def assemble(results):
    blocks = [results[c]["out"] for c in range(N_CORES)]
    return np.concatenate(blocks, 0).reshape(B, S, D).astype(np.float32)


def kernel(query, key, value, mask, wq, wk, wv, wo):
    # mask is all-False in this problem: softmax without masking.
    nc = _get_nc()
    in_maps = make_in_maps(query, key, value, wq, wk, wv, wo)
    from concourse.bass_utils import run_bass_kernel_spmd

    res = run_bass_kernel_spmd(nc, in_maps, list(range(N_CORES)))
    return assemble(res.results)


# revision 10
# speedup vs baseline: 1.2144x; 1.2144x over previous
"""MHA on 8 NeuronCores, v3: pipelined projections + row-tiled scores.

Core c owns token block c = (batch c//2, seq half c%2), 1024 tokens.
Pair p = heads (2p, 2p+1): K^T/Q^T d-rows 128p..128(p+1) in one tile
(head-even rows 0-63, head-odd rows 64-127).

  - Projections emitted per d-block (K_p, Q_p) / per (token-chunk, half)
    (V), interleaved with attention so ACT's exp stream starts ~10us in.
  - K AllGathers per 128-row block (8 pairwise AGs), V per token quarter
    (4 AGs). All attention K/V inputs are read from AG outputs so the SPMD
    program is core-symmetric (keys in global order).
  - Scores use 64x128 PE row tiling: head-even at tile_position (0,0),
    head-odd at (64,0) -> both heads stream concurrently, recovering the
    2x lost to the 64-deep head_dim contraction.
  - exp on ACT per [128, 2, 512] PSUM group tile (scale=1/8) -> pg bf16.
  - PV per head: 16 x [128k,65]x[128k,512] chained matmuls; 65th V row of
    ones gives the softmax denominator. PV groups are woven between scores
    groups via a queue so the PE alternates and ACT never starves.
  - Normalization via DRAM-bounce reciprocal broadcast, per (pair, qb).
  - wo emitted per query-half so most overlaps the last pair's attention.
"""
import numpy as np
import ml_dtypes
from collections import deque

import concourse.bass as bass
import concourse.bacc as bacc
import concourse.tile as tile
import concourse.mybir as mybir

N_CORES = 8
P = 128
B, S, D = 4, 2048, 1024
TOK = 1024
CD = D // P  # 8 d-blocks
QB = 512
NKC = S // P  # 16 key chunks (global order)
NG = 8  # groups of 2 key chunks
F32 = mybir.dt.float32
BF16 = mybir.dt.bfloat16
EXP = mybir.ActivationFunctionType.Exp
PAIR_GROUPS = [[2 * i, 2 * i + 1] for i in range(4)]

_CACHE = {}


def _n_excess_waits(nc):
    import json

    m = json.loads(nc.to_json_bytes())
    insts = [i for f in m["functions"] for b in f["blocks"] for i in b["instructions"]]
    return sum(
        1
        for i in insts
        if len((i.get("sync_info") or {}).get("on_wait", [])) >= 2
        and i.get("opcode") != "EventSemaphore"
    )


def _finish(nc):
    nc.compile()
    import bass_rust

    for _ in range(6):
        if _n_excess_waits(nc) == 0:
            break
        bass_rust.generate_event_semaphores(nc)
    assert _n_excess_waits(nc) == 0, "excess sync waits remain"
    nc.codegen_inst_isa_subclasses()
    return nc


def build_nc(scopes=False):
    nc = bacc.Bacc("TRN2", target_bir_lowering=False, debug=False, num_devices=N_CORES)

    xqT_d = nc.dram_tensor("xqT", [D, TOK], BF16, kind="ExternalInput").ap()
    xkT_d = nc.dram_tensor("xkT", [D, TOK], BF16, kind="ExternalInput").ap()
    xvT_d = nc.dram_tensor("xvT", [D, TOK], BF16, kind="ExternalInput").ap()
    wqkvT = nc.dram_tensor("wqkvT", [D, 3 * D], BF16, kind="ExternalInput").ap()
    woT = nc.dram_tensor("woT", [D, D], BF16, kind="ExternalInput").ap()
    out = nc.dram_tensor("out", [TOK, D], F32, kind="ExternalOutput").ap()

    kag_i = nc.dram_tensor("kag_i", [D, TOK], BF16).ap()
    kag_os = [nc.dram_tensor(f"kag_o{p}", [2, P, TOK], BF16).ap() for p in range(CD)]
    vag_i = nc.dram_tensor("vag_i", [TOK, D], BF16).ap()
    vag_os = [
        nc.dram_tensor(f"vag_o{q}", [2, TOK // 4, D], BF16).ap() for q in range(4)
    ]
    warm_i = nc.dram_tensor("warm_i", [1, 64], BF16).ap()
    warm_o = nc.dram_tensor("warm_o", [2, 1, 64], BF16).ap()
    den_d = nc.dram_tensor("den_d", [16, TOK], F32).ap()
    recip_d = nc.dram_tensor("recip_d", [16, TOK], F32).ap()

    from contextlib import ExitStack, nullcontext

    def scope(name):
        return nc.named_scope(name) if scopes else nullcontext()

    AG_KW = dict(
        kind="AllGather", op=mybir.AluOpType.bypass, replica_groups=PAIR_GROUPS
    )

    with tile.TileContext(nc) as tc:
        persist = ExitStack()
        ktp = persist.enter_context(tc.tile_pool(name="ktp", bufs=3))
        qtp = persist.enter_context(tc.tile_pool(name="qtp", bufs=3))
        vtp = persist.enter_context(tc.tile_pool(name="vtp", bufs=3))
        ltp = persist.enter_context(tc.tile_pool(name="ltp", bufs=1))
        pgp = persist.enter_context(tc.tile_pool(name="pgp", bufs=14))
        arp = persist.enter_context(tc.tile_pool(name="arp", bufs=2))
        obp = persist.enter_context(tc.tile_pool(name="obp", bufs=1))
        nrm = persist.enter_context(tc.tile_pool(name="nrm", bufs=1))
        s_ps = persist.enter_context(tc.tile_pool(name="s_ps", bufs=2, space="PSUM"))
        pv_ps = persist.enter_context(tc.tile_pool(name="pv_ps", bufs=2, space="PSUM"))
        mm_ps = persist.enter_context(tc.tile_pool(name="mm_ps", bufs=2, space="PSUM"))
        pha = ExitStack()
        xp = pha.enter_context(tc.tile_pool(name="xp", bufs=1))
        wkq = pha.enter_context(tc.tile_pool(name="wkq", bufs=1))
        wvp = pha.enter_context(tc.tile_pool(name="wvp", bufs=1))
        evp = pha.enter_context(tc.tile_pool(name="evp", bufs=3))

        # ---------------- input loads ----------------
        # order matters: proj_k(0) inputs first, then a dummy AG to pay the
        # one-time collective-init cost during the load phase.
        with scope("load"):
            nc.gpsimd.collective_compute(ins=[warm_i[:]], outs=[warm_o[:]], **AG_KW)
            xkT, xqT, xvT = [], [], []
            wk_b, wq_b, wv_j = [], [], []
            for j in range(CD):
                t = xp.tile([P, TOK], BF16, name=f"xkT_{j}")
                nc.sync.dma_start(out=t, in_=xkT_d[j * P : (j + 1) * P, :])
                xkT.append(t)
            for kind, base, lst in (("k", D, wk_b), ("q", 0, wq_b)):
                for p in range(CD):
                    t = wkq.tile([P, CD, P], BF16, name=f"w{kind}b_{p}")
                    ap_src = bass.AP(
                        tensor=wqkvT.tensor,
                        offset=base + p * P,
                        ap=[[3 * D, P], [P * 3 * D, CD], [1, P]],
                    )
                    nc.sync.dma_start(out=t, in_=ap_src)
                    lst.append(t)
            for j in range(CD):
                t = xp.tile([P, TOK], BF16, name=f"xqT_{j}")
                nc.sync.dma_start(out=t, in_=xqT_d[j * P : (j + 1) * P, :])
                xqT.append(t)
            for j in range(CD):
                t = xp.tile([P, TOK], BF16, name=f"xvT_{j}")
                nc.sync.dma_start(out=t, in_=xvT_d[j * P : (j + 1) * P, :])
                xvT.append(t)
            for j in range(CD):
                t = wvp.tile([P, D], BF16, name=f"wv_{j}")
                nc.sync.dma_start(out=t, in_=wqkvT[j * P : (j + 1) * P, 2 * D : 3 * D])
                wv_j.append(t)

        lts = [ltp.tile([P, TOK], BF16, name=f"lt_{i}") for i in range(CD)]

        kT_t = [None] * CD
        qT_t = [None] * CD
        v_t = [None] * CD  # per pair: [128 r, 16 kc, 2 hh, 65]

        # ---------------- emission units ----------------
        def proj_k(p):
            with scope(f"proj_k{p}"):
                for half in range(2):
                    ps = mm_ps.tile([P, QB], F32, name="ps_k", tag="mmps")
                    for j in range(CD):
                        nc.tensor.matmul(
                            ps,
                            wk_b[p][:, j, :],
                            xkT[j][:, half * QB : (half + 1) * QB],
                            start=(j == 0),
                            stop=(j == CD - 1),
                        )
                    sb = evp.tile([P, QB], BF16, name="sb_k", tag="evp")
                    nc.vector.tensor_copy(sb, ps)
                    nc.sync.dma_start(
                        out=kag_i[p * P : (p + 1) * P, half * QB : (half + 1) * QB],
                        in_=sb,
                    )
            with scope(f"ag_k{p}"):
                nc.gpsimd.collective_compute(
                    ins=[kag_i[p * P : (p + 1) * P, :]], outs=[kag_os[p][:]], **AG_KW
                )
            t = ktp.tile([P, S], BF16, name=f"kT_{p}", tag="ktp")
            nc.sync.dma_start(out=t[:, 0:TOK], in_=kag_os[p][0])
            nc.sync.dma_start(out=t[:, TOK:S], in_=kag_os[p][1])
            kT_t[p] = t

        def proj_q(p):
            with scope(f"proj_q{p}"):
                qt = qtp.tile([P, TOK], BF16, name=f"qT_{p}", tag="qtp")
                for half in range(2):
                    ps = mm_ps.tile([P, QB], F32, name="ps_q", tag="mmps")
                    for j in range(CD):
                        nc.tensor.matmul(
                            ps,
                            wq_b[p][:, j, :],
                            xqT[j][:, half * QB : (half + 1) * QB],
                            start=(j == 0),
                            stop=(j == CD - 1),
                        )
                    nc.vector.tensor_copy(qt[:, half * QB : (half + 1) * QB], ps)
                qT_t[p] = qt

        def proj_v(t_i, half):
            with scope(f"proj_v{t_i}_{half}"):
                ps = mm_ps.tile([P, QB], F32, name="ps_v", tag="mmps")
                for j in range(CD):
                    nc.tensor.matmul(
                        ps,
                        xvT[j][:, t_i * P : (t_i + 1) * P],
                        wv_j[j][:, half * QB : (half + 1) * QB],
                        start=(j == 0),
                        stop=(j == CD - 1),
                    )
                sb = evp.tile([P, QB], BF16, name="sb_v", tag="evp")
                nc.vector.tensor_copy(sb, ps)
                nc.sync.dma_start(
                    out=vag_i[t_i * P : (t_i + 1) * P, half * QB : (half + 1) * QB],
                    in_=sb,
                )

        def ag_v(q):
            with scope(f"ag_v{q}"):
                nc.gpsimd.collective_compute(
                    ins=[vag_i[q * TOK // 4 : (q + 1) * TOK // 4, :]],
                    outs=[vag_os[q][:]],
                    **AG_KW,
                )

        def v_fill(p):
            # v_t[p][r, kc, hh, c] = V[key kc*128+r, head dim 64*(2p+hh)+c]
            # global chunk kc = half*8 + 2q + i from vag_os[q][half, i*128+r]
            t = vtp.tile([P, NKC, 2, 65], BF16, name=f"v_{p}", tag="vtp")
            nc.vector.memset(t[:, :, :, 64:65], 1.0)
            for q in range(4):
                for half in range(2):
                    for hh in range(2):
                        src = bass.AP(
                            tensor=vag_os[q].tensor,
                            offset=vag_os[q].offset
                            + (half * (TOK // 4)) * D
                            + 128 * p
                            + 64 * hh,
                            ap=[[D, P], [P * D, 2], [1, 64]],
                        )
                        nc.sync.dma_start(
                            out=t[:, half * 8 + 2 * q : half * 8 + 2 * q + 2, hh, 0:64],
                            in_=src,
                        )
            v_t[p] = t

        # -------- attention --------
        pg_tiles = {}
        pvs_tiles = {}

        def sc_exp(p, qb, kc):
            # one key chunk, both heads: T0 -> sg[:,0,:], T8 -> sg[:,1,:];
            # a single exp frees both, so the row-tiled pair runs concurrently
            qs = slice(qb * QB, (qb + 1) * QB)
            ks = slice(kc * P, (kc + 1) * P)
            with scope(f"sc{p}_{qb}_{kc}"):
                sg = s_ps.tile([P, 2, QB], F32, name="sg", tag="s_ps")
                nc.tensor.matmul(
                    sg[:, 0, :], kT_t[p][0:64, ks], qT_t[p][0:64, qs],
                    start=True, stop=True, tile_position=(0, 0),
                )
                nc.tensor.matmul(
                    sg[:, 1, :], kT_t[p][64:P, ks], qT_t[p][64:P, qs],
                    start=True, stop=True, tile_position=(64, 0),
                )
                pg = pgp.tile([P, 2, QB], BF16, name="pg", tag="pgp")
                nc.scalar.activation(pg, sg, EXP, scale=0.125)
                pg_tiles[(p, qb, kc)] = pg

        def pv_part(p, qb, kc):
            if kc == 0:
                pvs_tiles[(p, qb)] = [
                    pv_ps.tile([65, QB], F32, name=f"pv{hh}", tag="pv_ps")
                    for hh in range(2)
                ]
            pvs = pvs_tiles[(p, qb)]
            pg = pg_tiles.pop((p, qb, kc))
            for hh in range(2):
                nc.tensor.matmul(
                    pvs[hh],
                    v_t[p][:, kc, hh, :],
                    pg[:, hh, :],
                    start=(kc == 0),
                    stop=(kc == NKC - 1),
                )

        def pv_fin(p, qb):
            with scope(f"fin{p}_{qb}"):
                pvs = pvs_tiles.pop((p, qb))
                ar = arp.tile([65, 2, QB], F32, name="ar", tag="arp")
                for hh in range(2):
                    nc.vector.tensor_copy(ar[:, hh, :], pvs[hh])
                off = 2 * p * TOK + qb * QB
                nc.sync.dma_start(
                    out=bass.AP(tensor=den_d.tensor, offset=off, ap=[[TOK, 2], [1, QB]]),
                    in_=ar[64:65, :, :],
                )
                dsq = nrm.tile([64, 16], F32, name="dsq", tag="dsq")
                nc.sync.dma_start(
                    out=dsq,
                    in_=bass.AP(
                        tensor=den_d.tensor, offset=off, ap=[[8, 64], [1, 8], [TOK, 2]]
                    ),
                )
                rsq = nrm.tile([64, 16], F32, name="rsq", tag="rsq")
                nc.vector.reciprocal(rsq, dsq)
                nc.sync.dma_start(
                    out=bass.AP(
                        tensor=recip_d.tensor, offset=off, ap=[[8, 64], [1, 8], [TOK, 2]]
                    ),
                    in_=rsq,
                )
                bc = nrm.tile([64, 2, QB], F32, name="bc", tag="bc")
                nc.sync.dma_start(
                    out=bc,
                    in_=bass.AP(
                        tensor=recip_d.tensor, offset=off, ap=[[0, 64], [TOK, 2], [1, QB]]
                    ),
                )
                for hh in range(2):
                    rr = slice(64 * hh, 64 * hh + 64)
                    nc.vector.tensor_mul(
                        lts[p][rr, qb * QB : (qb + 1) * QB], ar[0:64, hh, :], bc[:, hh, :]
                    )

        wo_t = [None] * CD

        def load_wo():
            for j in range(CD):
                t = ltp.tile([P, D], BF16, name=f"wo_{j}")
                nc.sync.dma_start(out=t, in_=woT[j * P : (j + 1) * P, :])
                wo_t[j] = t

        def wo_unit(t_i):
            with scope(f"wo{t_i}"):
                ob = obp.tile([P, D], F32, name="ob", tag="ob")
                for hh in range(2):
                    ps3 = mm_ps.tile([P, QB], F32, name="ps3", tag="mmps")
                    for sc_i in range(CD):
                        nc.tensor.matmul(
                            ps3,
                            lts[sc_i][:, t_i * P : (t_i + 1) * P],
                            wo_t[sc_i][:, hh * QB : (hh + 1) * QB],
                            start=(sc_i == 0),
                            stop=(sc_i == CD - 1),
                        )
                    nc.vector.tensor_copy(ob[:, hh * QB : (hh + 1) * QB], ps3)
                nc.sync.dma_start(out=out[t_i * P : (t_i + 1) * P, :], in_=ob)

        # ---------------- master emission sequence ----------------
        # pv work queue: woven between scores groups to keep both PE and ACT fed
        pvq = deque()

        def queue_pv(p, qb):
            for kc in range(NKC):
                pvq.append(lambda p=p, qb=qb, kc=kc: pv_part(p, qb, kc))
            pvq.append(lambda p=p, qb=qb: pv_fin(p, qb))

        def drain(n):
            for _ in range(min(n, len(pvq))):
                pvq.popleft()()

        # drain pv parts in pairs every other chunk to halve PE tiling-mode switches
        DS = [0, 2, 0, 2, 0, 2, 0, 2, 0, 2, 0, 2, 0, 2, 0, 3]

        def unit(p, qb):
            for kc in range(NKC):
                sc_exp(p, qb, kc)
                drain(DS[kc])

        proj_k(0); proj_q(0)
        # pair-0 qb0 scores stream while all V projections run on the PE
        pv_units = [(t, h) for h in range(2) for t in range(8)]
        for kc in range(NKC):
            sc_exp(0, 0, kc)
            proj_v(*pv_units[kc])
        for q in range(4):
            ag_v(q)
        proj_k(1); proj_q(1)
        v_fill(0); v_fill(1)
        queue_pv(0, 0)
        unit(0, 1)
        queue_pv(0, 1); proj_k(2); proj_q(2); v_fill(2)
        unit(1, 0)
        queue_pv(1, 0); proj_k(3); proj_q(3); v_fill(3)
        unit(1, 1)
        queue_pv(1, 1); load_wo()
        unit(2, 0)
        queue_pv(2, 0); proj_k(4); proj_q(4); v_fill(4)
        unit(2, 1)
        queue_pv(2, 1)
        unit(3, 0)
        queue_pv(3, 0); proj_k(5); proj_q(5); v_fill(5)
        unit(3, 1)
        queue_pv(3, 1)
        unit(4, 0)
        queue_pv(4, 0); proj_k(6); proj_q(6); v_fill(6)
        unit(4, 1)
        queue_pv(4, 1)
        unit(5, 0)
        queue_pv(5, 0); proj_k(7); proj_q(7); v_fill(7)
        unit(5, 1)
        queue_pv(5, 1)
        pha.close()
        unit(6, 0)
        queue_pv(6, 0)
        unit(6, 1)
        queue_pv(6, 1)
        unit(7, 0)
        queue_pv(7, 0)
        unit(7, 1)
        queue_pv(7, 1)
        drain(9)
        wo_unit(0); wo_unit(1)
        drain(9)
        wo_unit(2); wo_unit(3)
        drain(len(pvq))
        for t_i in range(4, 8):
            wo_unit(t_i)

        persist.close()

    return _finish(nc)


def _get_nc(scopes=False):
    key = ("nc", scopes)
    if key not in _CACHE:
        _CACHE[key] = build_nc(scopes)
    return _CACHE[key]


def make_in_maps(query, key, value, wq, wk, wv, wo):
    qf = np.asarray(query, np.float32).reshape(B * S, D)
    kf = np.asarray(key, np.float32).reshape(B * S, D)
    vf = np.asarray(value, np.float32).reshape(B * S, D)
    wqkvT = np.ascontiguousarray(
        np.concatenate([np.asarray(wq), np.asarray(wk), np.asarray(wv)], 0).T
    ).astype(ml_dtypes.bfloat16)
    woT_h = np.ascontiguousarray(np.asarray(wo).T).astype(ml_dtypes.bfloat16)
    in_maps = []
    for c in range(N_CORES):
        sl = slice(c * TOK, (c + 1) * TOK)
        in_maps.append(
            {
                "xqT": np.ascontiguousarray(qf[sl].T).astype(ml_dtypes.bfloat16),
                "xkT": np.ascontiguousarray(kf[sl].T).astype(ml_dtypes.bfloat16),
                "xvT": np.ascontiguousarray(vf[sl].T).astype(ml_dtypes.bfloat16),
                "wqkvT": wqkvT,
                "woT": woT_h,
            }
        )
    return in_maps


def assemble(results):
    blocks = [results[c]["out"] for c in range(N_CORES)]
    return np.concatenate(blocks, 0).reshape(B, S, D).astype(np.float32)


def kernel(query, key, value, mask, wq, wk, wv, wo):
    # mask is all-False in this problem: softmax without masking.
    nc = _get_nc()
    in_maps = make_in_maps(query, key, value, wq, wk, wv, wo)
    from concourse.bass_utils import run_bass_kernel_spmd

    res = run_bass_kernel_spmd(nc, in_maps, list(range(N_CORES)))
    return assemble(res.results)


# revision 13
# speedup vs baseline: 1.2690x; 1.0450x over previous
"""MHA on 8 NeuronCores, v3: pipelined projections + row-tiled scores.

Core c owns token block c = (batch c//2, seq half c%2), 1024 tokens.
Pair p = heads (2p, 2p+1): K^T/Q^T d-rows 128p..128(p+1) in one tile
(head-even rows 0-63, head-odd rows 64-127).

  - Projections emitted per d-block (K_p, Q_p) / per (token-chunk, half)
    (V), interleaved with attention so ACT's exp stream starts ~10us in.
  - K AllGathers per 128-row block (8 pairwise AGs), V per token quarter
    (4 AGs). All attention K/V inputs are read from AG outputs so the SPMD
    program is core-symmetric (keys in global order).
  - Scores use 64x128 PE row tiling: head-even at tile_position (0,0),
    head-odd at (64,0) -> both heads stream concurrently, recovering the
    2x lost to the 64-deep head_dim contraction.
  - exp on ACT per [128, 2, 512] PSUM group tile (scale=1/8) -> pg bf16.
  - PV per head: 16 x [128k,65]x[128k,512] chained matmuls; 65th V row of
    ones gives the softmax denominator. PV groups are woven between scores
    groups via a queue so the PE alternates and ACT never starves.
  - Normalization via DRAM-bounce reciprocal broadcast, per (pair, qb).
  - wo emitted per query-half so most overlaps the last pair's attention.
"""
import numpy as np
import ml_dtypes
from collections import deque

import concourse.bass as bass
import concourse.bacc as bacc
import concourse.tile as tile
import concourse.mybir as mybir

N_CORES = 8
P = 128
B, S, D = 4, 2048, 1024
TOK = 1024
CD = D // P  # 8 d-blocks
QB = 512
NKC = S // P  # 16 key chunks (global order)
NG = 8  # groups of 2 key chunks
F32 = mybir.dt.float32
BF16 = mybir.dt.bfloat16
EXP = mybir.ActivationFunctionType.Exp
PAIR_GROUPS = [[2 * i, 2 * i + 1] for i in range(4)]

_CACHE = {}


def _n_excess_waits(nc):
    import json

    m = json.loads(nc.to_json_bytes())
    insts = [i for f in m["functions"] for b in f["blocks"] for i in b["instructions"]]
    return sum(
        1
        for i in insts
        if len((i.get("sync_info") or {}).get("on_wait", [])) >= 2
        and i.get("opcode") != "EventSemaphore"
    )


def _finish(nc):
    nc.compile()
    import bass_rust

    for _ in range(6):
        if _n_excess_waits(nc) == 0:
            break
        bass_rust.generate_event_semaphores(nc)
    assert _n_excess_waits(nc) == 0, "excess sync waits remain"
    nc.codegen_inst_isa_subclasses()
    return nc


def build_nc(scopes=False):
    nc = bacc.Bacc("TRN2", target_bir_lowering=False, debug=False, num_devices=N_CORES)

    xqT_d = nc.dram_tensor("xqT", [D, TOK], BF16, kind="ExternalInput").ap()
    xkT_d = nc.dram_tensor("xkT", [D, TOK], BF16, kind="ExternalInput").ap()
    xvT_d = nc.dram_tensor("xvT", [D, TOK], BF16, kind="ExternalInput").ap()
    wqkvT = nc.dram_tensor("wqkvT", [D, 3 * D], BF16, kind="ExternalInput").ap()
    woT = nc.dram_tensor("woT", [D, D], BF16, kind="ExternalInput").ap()
    out = nc.dram_tensor("out", [TOK, D], F32, kind="ExternalOutput").ap()

    kag_i = nc.dram_tensor("kag_i", [D, TOK], BF16).ap()
    kag_os = [nc.dram_tensor(f"kag_o{p}", [2, P, TOK], BF16).ap() for p in range(CD)]
    vag_i = nc.dram_tensor("vag_i", [TOK, D], BF16).ap()
    vag_os = [
        nc.dram_tensor(f"vag_o{q}", [2, TOK // 4, D], BF16).ap() for q in range(4)
    ]
    warm_i = nc.dram_tensor("warm_i", [1, 64], BF16).ap()
    warm_o = nc.dram_tensor("warm_o", [2, 1, 64], BF16).ap()
    den_d = nc.dram_tensor("den_d", [16, TOK], F32).ap()
    recip_d = nc.dram_tensor("recip_d", [16, TOK], F32).ap()

    from contextlib import ExitStack, nullcontext

    def scope(name):
        return nc.named_scope(name) if scopes else nullcontext()

    AG_KW = dict(
        kind="AllGather", op=mybir.AluOpType.bypass, replica_groups=PAIR_GROUPS
    )

    with tile.TileContext(nc) as tc:
        persist = ExitStack()
        ktp = persist.enter_context(tc.tile_pool(name="ktp", bufs=3))
        qtp = persist.enter_context(tc.tile_pool(name="qtp", bufs=3))
        vtp = persist.enter_context(tc.tile_pool(name="vtp", bufs=3))
        ltp = persist.enter_context(tc.tile_pool(name="ltp", bufs=1))
        pgp = persist.enter_context(tc.tile_pool(name="pgp", bufs=13))
        arp = persist.enter_context(tc.tile_pool(name="arp", bufs=2))
        obp = persist.enter_context(tc.tile_pool(name="obp", bufs=1))
        nrm = persist.enter_context(tc.tile_pool(name="nrm", bufs=2))
        s_ps = persist.enter_context(tc.tile_pool(name="s_ps", bufs=2, space="PSUM"))
        pv_ps = persist.enter_context(tc.tile_pool(name="pv_ps", bufs=2, space="PSUM"))
        mm_ps = persist.enter_context(tc.tile_pool(name="mm_ps", bufs=2, space="PSUM"))
        pha = ExitStack()
        xp = pha.enter_context(tc.tile_pool(name="xp", bufs=1))
        wkq = pha.enter_context(tc.tile_pool(name="wkq", bufs=1))
        wvp = pha.enter_context(tc.tile_pool(name="wvp", bufs=1))
        evp = pha.enter_context(tc.tile_pool(name="evp", bufs=3))

        # ---------------- input loads ----------------
        # order matters: proj_k(0) inputs first, then a dummy AG to pay the
        # one-time collective-init cost during the load phase.
        with scope("load"):
            nc.gpsimd.collective_compute(ins=[warm_i[:]], outs=[warm_o[:]], **AG_KW)
            xkT, xqT, xvT = [], [], []
            wk_b, wq_b, wv_j = [], [], []
            for j in range(CD):
                t = xp.tile([P, TOK], BF16, name=f"xkT_{j}")
                nc.sync.dma_start(out=t, in_=xkT_d[j * P : (j + 1) * P, :])
                xkT.append(t)
            for kind, base, lst in (("k", D, wk_b), ("q", 0, wq_b)):
                for p in range(CD):
                    t = wkq.tile([P, CD, P], BF16, name=f"w{kind}b_{p}")
                    ap_src = bass.AP(
                        tensor=wqkvT.tensor,
                        offset=base + p * P,
                        ap=[[3 * D, P], [P * 3 * D, CD], [1, P]],
                    )
                    nc.sync.dma_start(out=t, in_=ap_src)
                    lst.append(t)
            for j in range(CD):
                t = xp.tile([P, TOK], BF16, name=f"xqT_{j}")
                nc.sync.dma_start(out=t, in_=xqT_d[j * P : (j + 1) * P, :])
                xqT.append(t)


        def load_v_inputs():
            for j in range(CD):
                t = xp.tile([P, TOK], BF16, name=f"xvT_{j}")
                nc.sync.dma_start(out=t, in_=xvT_d[j * P : (j + 1) * P, :])
                xvT.append(t)
            for j in range(CD):
                t = wvp.tile([P, D], BF16, name=f"wv_{j}")
                nc.sync.dma_start(out=t, in_=wqkvT[j * P : (j + 1) * P, 2 * D : 3 * D])
                wv_j.append(t)

        lts = [ltp.tile([P, TOK], BF16, name=f"lt_{i}") for i in range(CD)]

        kT_t = [None] * CD
        qT_t = [None] * CD
        v_t = [None] * CD  # per pair: [128 r, 16 kc, 2 hh, 65]

        # ---------------- emission units ----------------
        def proj_k(p):
            with scope(f"proj_k{p}"):
                for half in range(2):
                    ps = mm_ps.tile([P, QB], F32, name="ps_k", tag="mmps")
                    for j in range(CD):
                        nc.tensor.matmul(
                            ps,
                            wk_b[p][:, j, :],
                            xkT[j][:, half * QB : (half + 1) * QB],
                            start=(j == 0),
                            stop=(j == CD - 1),
                        )
                    sb = evp.tile([P, QB], BF16, name="sb_k", tag="evp")
                    nc.vector.tensor_copy(sb, ps)
                    nc.sync.dma_start(
                        out=kag_i[p * P : (p + 1) * P, half * QB : (half + 1) * QB],
                        in_=sb,
                    )
            with scope(f"ag_k{p}"):
                nc.gpsimd.collective_compute(
                    ins=[kag_i[p * P : (p + 1) * P, :]], outs=[kag_os[p][:]], **AG_KW
                )
            t = ktp.tile([P, S], BF16, name=f"kT_{p}", tag="ktp")
            nc.sync.dma_start(out=t[:, 0:TOK], in_=kag_os[p][0])
            nc.sync.dma_start(out=t[:, TOK:S], in_=kag_os[p][1])
            kT_t[p] = t

        def proj_q(p):
            with scope(f"proj_q{p}"):
                qt = qtp.tile([P, TOK], BF16, name=f"qT_{p}", tag="qtp")
                for half in range(2):
                    ps = mm_ps.tile([P, QB], F32, name="ps_q", tag="mmps")
                    for j in range(CD):
                        nc.tensor.matmul(
                            ps,
                            wq_b[p][:, j, :],
                            xqT[j][:, half * QB : (half + 1) * QB],
                            start=(j == 0),
                            stop=(j == CD - 1),
                        )
                    nc.vector.tensor_copy(qt[:, half * QB : (half + 1) * QB], ps)
                qT_t[p] = qt

        def proj_v(t_i, half):
            with scope(f"proj_v{t_i}_{half}"):
                ps = mm_ps.tile([P, QB], F32, name="ps_v", tag="mmps")
                for j in range(CD):
                    nc.tensor.matmul(
                        ps,
                        xvT[j][:, t_i * P : (t_i + 1) * P],
                        wv_j[j][:, half * QB : (half + 1) * QB],
                        start=(j == 0),
                        stop=(j == CD - 1),
                    )
                sb = evp.tile([P, QB], BF16, name="sb_v", tag="evp")
                nc.vector.tensor_copy(sb, ps)
                nc.sync.dma_start(
                    out=vag_i[t_i * P : (t_i + 1) * P, half * QB : (half + 1) * QB],
                    in_=sb,
                )

        def ag_v(q):
            with scope(f"ag_v{q}"):
                nc.gpsimd.collective_compute(
                    ins=[vag_i[q * TOK // 4 : (q + 1) * TOK // 4, :]],
                    outs=[vag_os[q][:]],
                    **AG_KW,
                )

        def v_fill(p):
            # v_t[p][r, kc, hh, c] = V[key kc*128+r, head dim 64*(2p+hh)+c]
            # global chunk kc = half*8 + 2q + i from vag_os[q][half, i*128+r]
            t = vtp.tile([P, NKC, 2, 65], BF16, name=f"v_{p}", tag="vtp")
            nc.vector.memset(t[:, :, :, 64:65], 1.0)
            for q in range(4):
                for half in range(2):
                    for hh in range(2):
                        src = bass.AP(
                            tensor=vag_os[q].tensor,
                            offset=vag_os[q].offset
                            + (half * (TOK // 4)) * D
                            + 128 * p
                            + 64 * hh,
                            ap=[[D, P], [P * D, 2], [1, 64]],
                        )
                        nc.sync.dma_start(
                            out=t[:, half * 8 + 2 * q : half * 8 + 2 * q + 2, hh, 0:64],
                            in_=src,
                        )
            v_t[p] = t

        # -------- attention --------
        pg_tiles = {}
        pvs_tiles = {}

        def sc_exp(p, qb, kc):
            # one key chunk, both heads: T0 -> sg[:,0,:], T8 -> sg[:,1,:];
            # a single exp frees both, so the row-tiled pair runs concurrently
            qs = slice(qb * QB, (qb + 1) * QB)
            ks = slice(kc * P, (kc + 1) * P)
            with scope(f"sc{p}_{qb}_{kc}"):
                sg = s_ps.tile([P, 2, QB], F32, name="sg", tag="s_ps")
                nc.tensor.matmul(
                    sg[:, 0, :], kT_t[p][0:64, ks], qT_t[p][0:64, qs],
                    start=True, stop=True, tile_position=(0, 0),
                )
                nc.tensor.matmul(
                    sg[:, 1, :], kT_t[p][64:P, ks], qT_t[p][64:P, qs],
                    start=True, stop=True, tile_position=(64, 0),
                )
                pg = pgp.tile([P, 2, QB], BF16, name="pg", tag="pgp")
                nc.scalar.activation(pg, sg, EXP, scale=0.125)
                pg_tiles[(p, qb, kc)] = pg

        def pv_part(p, qb, kc):
            if kc == 0:
                pvs_tiles[(p, qb)] = [
                    pv_ps.tile([65, QB], F32, name=f"pv{hh}", tag="pv_ps")
                    for hh in range(2)
                ]
            pvs = pvs_tiles[(p, qb)]
            pg = pg_tiles.pop((p, qb, kc))
            for hh in range(2):
                nc.tensor.matmul(
                    pvs[hh],
                    v_t[p][:, kc, hh, :],
                    pg[:, hh, :],
                    start=(kc == 0),
                    stop=(kc == NKC - 1),
                )

        ar_tiles = {}
        bc_tiles = {}

        def pv_fin(p, qb):
            with scope(f"fin{p}_{qb}"):
                pvs = pvs_tiles.pop((p, qb))
                ar = arp.tile([65, 2, QB], F32, name="ar", tag="arp")
                ar_tiles[(p, qb)] = ar
                for hh in range(2):
                    nc.vector.tensor_copy(ar[:, hh, :], pvs[hh])
                off = 2 * p * TOK + qb * QB
                nc.sync.dma_start(
                    out=bass.AP(tensor=den_d.tensor, offset=off, ap=[[TOK, 2], [1, QB]]),
                    in_=ar[64:65, :, :],
                )
                dsq = nrm.tile([64, 16], F32, name="dsq", tag="dsq")
                nc.sync.dma_start(
                    out=dsq,
                    in_=bass.AP(
                        tensor=den_d.tensor, offset=off, ap=[[8, 64], [1, 8], [TOK, 2]]
                    ),
                )
                rsq = nrm.tile([64, 16], F32, name="rsq", tag="rsq")
                nc.vector.reciprocal(rsq, dsq)
                nc.sync.dma_start(
                    out=bass.AP(
                        tensor=recip_d.tensor, offset=off, ap=[[8, 64], [1, 8], [TOK, 2]]
                    ),
                    in_=rsq,
                )
                bc = nrm.tile([64, 2, QB], F32, name="bc", tag="bc")
                bc_tiles[(p, qb)] = bc
                nc.sync.dma_start(
                    out=bc,
                    in_=bass.AP(
                        tensor=recip_d.tensor, offset=off, ap=[[0, 64], [TOK, 2], [1, QB]]
                    ),
                )

        def pv_fin_b(p, qb):
            # deferred one unit so the DVE never stalls on the DRAM bounce
            ar = ar_tiles.pop((p, qb))
            bc = bc_tiles.pop((p, qb))
            with scope(f"finb{p}_{qb}"):
                for hh in range(2):
                    rr = slice(64 * hh, 64 * hh + 64)
                    nc.vector.tensor_mul(
                        lts[p][rr, qb * QB : (qb + 1) * QB], ar[0:64, hh, :], bc[:, hh, :]
                    )

        wo_t = [None] * CD

        def load_wo():
            for j in range(CD):
                t = ltp.tile([P, D], BF16, name=f"wo_{j}")
                nc.sync.dma_start(out=t, in_=woT[j * P : (j + 1) * P, :])
                wo_t[j] = t

        def wo_unit(t_i):
            with scope(f"wo{t_i}"):
                ob = obp.tile([P, D], F32, name="ob", tag="ob")
                for hh in range(2):
                    ps3 = mm_ps.tile([P, QB], F32, name="ps3", tag="mmps")
                    for sc_i in range(CD):
                        nc.tensor.matmul(
                            ps3,
                            lts[sc_i][:, t_i * P : (t_i + 1) * P],
                            wo_t[sc_i][:, hh * QB : (hh + 1) * QB],
                            start=(sc_i == 0),
                            stop=(sc_i == CD - 1),
                        )
                    nc.vector.tensor_copy(ob[:, hh * QB : (hh + 1) * QB], ps3)
                nc.sync.dma_start(out=out[t_i * P : (t_i + 1) * P, :], in_=ob)

        # ---------------- master emission sequence ----------------
        # pv work queue: woven between scores groups to keep both PE and ACT fed
        pvq = deque()

        prev_fin = [None]

        def queue_pv(p, qb):
            if prev_fin[0] is not None:
                pf = prev_fin[0]
                pvq.append(lambda: pv_fin_b(*pf))
            prev_fin[0] = (p, qb)
            for kc in range(NKC):
                pvq.append(lambda p=p, qb=qb, kc=kc: pv_part(p, qb, kc))
            pvq.append(lambda p=p, qb=qb: pv_fin(p, qb))

        def drain(n):
            for _ in range(min(n, len(pvq))):
                pvq.popleft()()

        # drain pv parts in pairs every other chunk to halve PE tiling-mode switches
        DS = [0, 2, 0, 2, 0, 2, 0, 2, 0, 2, 0, 2, 0, 2, 0, 4]

        def unit(p, qb):
            for kc in range(NKC):
                sc_exp(p, qb, kc)
                drain(DS[kc])

        proj_k(0); proj_q(0)
        load_v_inputs()
        # pair-0 qb0 scores stream while all V projections run on the PE
        pv_units = [(t, h) for h in range(2) for t in range(8)]
        for kc in range(NKC):
            sc_exp(0, 0, kc)
            proj_v(*pv_units[kc])
        for q in range(4):
            ag_v(q)
        proj_k(1); proj_q(1)
        v_fill(0); v_fill(1)
        queue_pv(0, 0)
        unit(0, 1)
        queue_pv(0, 1); proj_k(2); proj_q(2); v_fill(2)
        unit(1, 0)
        queue_pv(1, 0); proj_k(3); proj_q(3); v_fill(3)
        unit(1, 1)
        queue_pv(1, 1); load_wo()
        unit(2, 0)
        queue_pv(2, 0); proj_k(4); proj_q(4); v_fill(4)
        unit(2, 1)
        queue_pv(2, 1)
        unit(3, 0)
        queue_pv(3, 0); proj_k(5); proj_q(5); v_fill(5)
        unit(3, 1)
        queue_pv(3, 1)
        unit(4, 0)
        queue_pv(4, 0); proj_k(6); proj_q(6); v_fill(6)
        unit(4, 1)
        queue_pv(4, 1)
        unit(5, 0)
        queue_pv(5, 0); proj_k(7); proj_q(7); v_fill(7)
        unit(5, 1)
        queue_pv(5, 1)
        pha.close()
        unit(6, 0)
        queue_pv(6, 0)
        unit(6, 1)
        queue_pv(6, 1)
        unit(7, 0)
        queue_pv(7, 0)
        unit(7, 1)
        queue_pv(7, 1)
        drain(9)
        wo_unit(0); wo_unit(1)
        drain(9)
        wo_unit(2); wo_unit(3)
        drain(len(pvq))
        pv_fin_b(*prev_fin[0])
        for t_i in range(4, 8):
            wo_unit(t_i)

        persist.close()

    return _finish(nc)


def _get_nc(scopes=False):
    key = ("nc", scopes)
    if key not in _CACHE:
        _CACHE[key] = build_nc(scopes)
    return _CACHE[key]


def make_in_maps(query, key, value, wq, wk, wv, wo):
    qf = np.asarray(query, np.float32).reshape(B * S, D)
    kf = np.asarray(key, np.float32).reshape(B * S, D)
    vf = np.asarray(value, np.float32).reshape(B * S, D)
    wqkvT = np.ascontiguousarray(
        np.concatenate([np.asarray(wq), np.asarray(wk), np.asarray(wv)], 0).T
    ).astype(ml_dtypes.bfloat16)
    woT_h = np.ascontiguousarray(np.asarray(wo).T).astype(ml_dtypes.bfloat16)
    in_maps = []
    for c in range(N_CORES):
        sl = slice(c * TOK, (c + 1) * TOK)
        in_maps.append(
            {
                "xqT": np.ascontiguousarray(qf[sl].T).astype(ml_dtypes.bfloat16),
                "xkT": np.ascontiguousarray(kf[sl].T).astype(ml_dtypes.bfloat16),
                "xvT": np.ascontiguousarray(vf[sl].T).astype(ml_dtypes.bfloat16),
                "wqkvT": wqkvT,
                "woT": woT_h,
            }
        )
    return in_maps


def assemble(results):
    blocks = [results[c]["out"] for c in range(N_CORES)]
    return np.concatenate(blocks, 0).reshape(B, S, D).astype(np.float32)


def kernel(query, key, value, mask, wq, wk, wv, wo):
    # mask is all-False in this problem: softmax without masking.
    nc = _get_nc()
    in_maps = make_in_maps(query, key, value, wq, wk, wv, wo)
    from concourse.bass_utils import run_bass_kernel_spmd

    res = run_bass_kernel_spmd(nc, in_maps, list(range(N_CORES)))
    return assemble(res.results)


# revision 14
# speedup vs baseline: 1.5012x; 1.1830x over previous
"""MHA on 8 NeuronCores, v2: query-token-sharded attention.

Core c owns token block c = (batch c//2, seq half c%2), 1024 tokens.
  - Phase 1 (token-parallel): Q^T, K^T, V for my block, all 1024 dims, bf16.
    Q^T never leaves SBUF. K^T and V go to pairwise AllGather (groups
    [2b, 2b+1]) so both cores of a batch hold the batch's full-sequence
    K^T [1024 d, 2048] and V [2048, 1024].
  - Phase 2: dense attention for MY 1024 queries x all 16 heads over the
    batch's 2048 keys. Scores transposed (S^T[k, q]) -> exp on ACT ->
    PV with a ones-row giving the softmax denominator for free; division
    via DRAM-broadcast reciprocal (reshaped [64, 16] so DVE reciprocal is
    cheap). Normalized A^T goes straight into SBUF tiles laid out for the
    output projection.
  - Phase 3: out = A @ wo^T for my tokens, entirely local. Host concatenates
    the 8 disjoint token blocks.

Only communication: 2 pairwise AllGathers (2MB in / 4MB out each),
fully overlapped with phase-1/2 compute. bf16 matmuls, fp32 PSUM.
"""
import numpy as np
import ml_dtypes

import concourse.bass as bass
import concourse.bacc as bacc
import concourse.tile as tile
import concourse.mybir as mybir

N_CORES = 8
P = 128
B, S, D = 4, 2048, 1024
TOK = 1024  # my tokens
CD = D // P
QB = 512
NKC = S // P  # 16 key chunks
F32 = mybir.dt.float32
BF16 = mybir.dt.bfloat16
EXP = mybir.ActivationFunctionType.Exp
PAIR_GROUPS = [[2 * i, 2 * i + 1] for i in range(4)]

_CACHE = {}


def _n_excess_waits(nc):
    import json

    m = json.loads(nc.to_json_bytes())
    insts = [i for f in m["functions"] for b in f["blocks"] for i in b["instructions"]]
    return sum(
        1
        for i in insts
        if len((i.get("sync_info") or {}).get("on_wait", [])) >= 2
        and i.get("opcode") != "EventSemaphore"
    )


def _finish(nc):
    nc.compile()
    import bass_rust

    for _ in range(6):
        if _n_excess_waits(nc) == 0:
            break
        bass_rust.generate_event_semaphores(nc)
    assert _n_excess_waits(nc) == 0, "excess sync waits remain"
    nc.codegen_inst_isa_subclasses()
    return nc


def build_nc(scopes=False):
    nc = bacc.Bacc("TRN2", target_bir_lowering=False, debug=False, num_devices=N_CORES)

    xqT_d = nc.dram_tensor("xqT", [D, TOK], BF16, kind="ExternalInput").ap()
    xkT_d = nc.dram_tensor("xkT", [D, TOK], BF16, kind="ExternalInput").ap()
    xvT_d = nc.dram_tensor("xvT", [D, TOK], BF16, kind="ExternalInput").ap()
    wqkvT = nc.dram_tensor("wqkvT", [D, 3 * D], BF16, kind="ExternalInput").ap()
    woT = nc.dram_tensor("woT", [D, D], BF16, kind="ExternalInput").ap()
    out = nc.dram_tensor("out", [TOK, D], F32, kind="ExternalOutput").ap()

    # pairwise exchange buffers
    kag_i = nc.dram_tensor("kag_i", [D, TOK], BF16).ap()
    kag_oA = nc.dram_tensor("kag_oA", [2, D // 2, TOK], BF16).ap()  # d-chunks 0-3
    kag_oB = nc.dram_tensor("kag_oB", [2, D // 2, TOK], BF16).ap()  # d-chunks 4-7
    vag_i = nc.dram_tensor("vag_i", [TOK, D], BF16).ap()
    vag_os = [
        nc.dram_tensor(f"vag_o{q}", [2, TOK // 4, D], BF16).ap() for q in range(4)
    ]
    den_d = nc.dram_tensor("den_d", [16, TOK], F32).ap()
    recip_d = nc.dram_tensor("recip_d", [16, TOK], F32).ap()

    from contextlib import ExitStack, nullcontext

    def scope(name):
        return nc.named_scope(name) if scopes else nullcontext()

    AG_KW = dict(
        kind="AllGather", op=mybir.AluOpType.bypass, replica_groups=PAIR_GROUPS
    )

    with tile.TileContext(nc) as tc:
        persist = ExitStack()
        qp = persist.enter_context(tc.tile_pool(name="qp", bufs=1))
        wop = persist.enter_context(tc.tile_pool(name="wop", bufs=1))
        ltp = persist.enter_context(tc.tile_pool(name="ltp", bufs=1))

        # ---------------- Phase 1: K, V (exchanged) then Q (stays local) ----
        with ExitStack() as ph1:
            xts = ph1.enter_context(tc.tile_pool(name="xts", bufs=1))
            wp = ph1.enter_context(tc.tile_pool(name="wp", bufs=1))
            ev1 = ph1.enter_context(tc.tile_pool(name="ev1", bufs=4))
            ps1 = ph1.enter_context(tc.tile_pool(name="ps1", bufs=3, space="PSUM"))

            with scope("load"):
                w_t, xqT, xkT, xvT = [], [], [], []
                # interleave w and xk loads so proj_k (first) starts ASAP
                for j in range(CD):
                    wt = wp.tile([P, 3 * D], BF16, name=f"w_{j}")
                    nc.sync.dma_start(out=wt, in_=wqkvT[j * P : (j + 1) * P, :])
                    w_t.append(wt)
                    t = xts.tile([P, TOK], BF16, name=f"xkT_{j}")
                    nc.sync.dma_start(out=t, in_=xkT_d[j * P : (j + 1) * P, :])
                    xkT.append(t)
                for nm, x, lst in (("v", xvT_d, xvT), ("q", xqT_d, xqT)):
                    for j in range(CD):
                        t = xts.tile([P, TOK], BF16, name=f"x{nm}T_{j}")
                        nc.sync.dma_start(out=t, in_=x[j * P : (j + 1) * P, :])
                        lst.append(t)

            # K^T [d-chunk, tok] -> kag_i
            with scope("proj_k"):
                for i in range(CD):
                    ps = ps1.tile([P, TOK], F32, name="ps_k", tag="ps1")
                    for j in range(CD):
                        lhsT = w_t[j][:, D + i * P : D + (i + 1) * P]
                        for h in range(TOK // QB):
                            nc.tensor.matmul(
                                ps[:, h * QB : (h + 1) * QB],
                                lhsT,
                                xkT[j][:, h * QB : (h + 1) * QB],
                                start=(j == 0),
                                stop=(j == CD - 1),
                            )
                    sb = ev1.tile([P, TOK], BF16, name="sb_k", tag="ev1")
                    (nc.scalar.copy if i % 2 == 0 else nc.vector.tensor_copy)(sb, ps)
                    nc.sync.dma_start(out=kag_i[i * P : (i + 1) * P, :], in_=sb)
            with scope("ag_k"):
                nc.gpsimd.collective_compute(
                    ins=[kag_i[0 : D // 2, :]], outs=[kag_oA[:]], **AG_KW
                )
                nc.gpsimd.collective_compute(
                    ins=[kag_i[D // 2 : D, :]], outs=[kag_oB[:]], **AG_KW
                )

            # Q^T [d-chunk, tok] -> SBUF (persistent)
            with scope("proj_q"):
                qT_t = []
                for i in range(CD):
                    ps = ps1.tile([P, TOK], F32, name="ps_q", tag="ps1")
                    for j in range(CD):
                        lhsT = w_t[j][:, i * P : (i + 1) * P]
                        for h in range(TOK // QB):
                            nc.tensor.matmul(
                                ps[:, h * QB : (h + 1) * QB],
                                lhsT,
                                xqT[j][:, h * QB : (h + 1) * QB],
                                start=(j == 0),
                                stop=(j == CD - 1),
                            )
                    qt = qp.tile([P, TOK], BF16, name=f"qT_{i}")
                    (nc.scalar.copy if i % 2 == 0 else nc.vector.tensor_copy)(qt, ps)
                    qT_t.append(qt)

            # V [tok-chunk, d] -> vag_i
            with scope("proj_v"):
                for t_i in range(CD):
                    ps = ps1.tile([P, D], F32, name="ps_v", tag="ps1")
                    for j in range(CD):
                        lhsT = xvT[j][:, t_i * P : (t_i + 1) * P]
                        for h in range(D // QB):
                            nc.tensor.matmul(
                                ps[:, h * QB : (h + 1) * QB],
                                lhsT,
                                w_t[j][:, 2 * D + h * QB : 2 * D + (h + 1) * QB],
                                start=(j == 0),
                                stop=(j == CD - 1),
                            )
                    sb = ev1.tile([P, D], BF16, name="sb_v", tag="ev1")
                    (nc.scalar.copy if t_i % 2 == 0 else nc.vector.tensor_copy)(sb, ps)
                    nc.sync.dma_start(out=vag_i[t_i * P : (t_i + 1) * P, :], in_=sb)
            with scope("ag_v"):
                for q in range(4):
                    nc.gpsimd.collective_compute(
                        ins=[vag_i[q * TOK // 4 : (q + 1) * TOK // 4, :]],
                        outs=[vag_os[q][:]],
                        **AG_KW,
                    )

        # ---------------- Phase 2: attention, 16 heads x my 1024 queries ----
        with ExitStack() as ph2:
            kst = ph2.enter_context(tc.tile_pool(name="kst", bufs=1))
            vp = ph2.enter_context(tc.tile_pool(name="vp", bufs=3))
            pt = ph2.enter_context(tc.tile_pool(name="pt", bufs=4))
            at = ph2.enter_context(tc.tile_pool(name="at", bufs=4))
            sm = ph2.enter_context(tc.tile_pool(name="sm", bufs=2))
            ps2 = ExitStack()
            s_ps = ps2.enter_context(tc.tile_pool(name="s_ps", bufs=2, space="PSUM"))
            pv_ps = ps2.enter_context(tc.tile_pool(name="pv_ps", bufs=2, space="PSUM"))

            # woT prefetch (phase 3) and lt output tiles
            wo_t = []
            for j in range(CD):
                wt3 = wop.tile([P, D], BF16, name=f"wo_{j}")
                nc.sync.dma_start(out=wt3, in_=woT[j * P : (j + 1) * P, :])
                wo_t.append(wt3)
            lts = [ltp.tile([P, TOK], BF16, name=f"lt_{i}") for i in range(CD)]

            # stage gathered K^T as 8 SBUF tiles [128 d-chunk, 2048 k]
            kT_s = []
            for j in range(CD):
                t = kst.tile([P, S], BF16, name=f"kTs_{j}")
                kg = kag_oA if j < 4 else kag_oB
                jj = j % 4
                nc.sync.dma_start(out=t[:, 0:TOK], in_=kg[0, jj * P : (jj + 1) * P, :])
                nc.sync.dma_start(out=t[:, TOK:S], in_=kg[1, jj * P : (jj + 1) * P, :])
                kT_s.append(t)

            KCS = [0, 1, 8, 9, 2, 3, 10, 11, 4, 5, 12, 13, 6, 7, 14, 15]
            GROUPS = [(0, 3), (3, 6), (6, 9), (9, 12), (12, 15), (15, 16)]

            # flatten (head, qblock, group) into a software pipeline with a
            # one-group scores lookahead so ACT(exp) never waits on PE latency
            units = []  # (h, qb) state
            vts, araws, pvs = {}, {}, {}

            def load_head(h):
                v_t = vp.tile([P, NKC, 65], BF16, name="v_t", tag="vp")
                for q in range(4):
                    for half in range(2):
                        vsrc = vag_os[q][half, :, 64 * h : 64 * h + 64]
                        nc.sync.dma_start(
                            out=v_t[:, 4 * q + 2 * half : 4 * q + 2 * half + 2, 0:64],
                            in_=vsrc.rearrange("(kc p) d -> p kc d", p=P),
                        )
                nc.vector.memset(v_t[:, :, 64:65], 1.0)
                vts[h] = v_t

            steps = [
                (h, qb, gi)
                for h in range(16)
                for qb in range(TOK // QB)
                for gi in range(len(GROUPS))
            ]

            def emit_scores(step):
                h, qb, gi = step
                if qb == 0 and gi == 0:
                    load_head(h)
                    araws[h] = at.tile([65, TOK], F32, name="a_raw", tag="at")
                g0, g1 = GROUPS[gi]
                if gi == 0:
                    pvs[(h, qb)] = pv_ps.tile([65, QB], F32, name="pv", tag="pv_ps")
                r = slice(64 * (h % 2), 64 * (h % 2) + 64)
                qs = slice(qb * QB, (qb + 1) * QB)
                sg = s_ps.tile([P, 3, QB], F32, name="sg", tag="s_ps")
                for pos in range(g0, g1):
                    kc = KCS[pos]
                    nc.tensor.matmul(
                        sg[:, pos - g0, :],
                        kT_s[h // 2][r, kc * P : (kc + 1) * P],
                        qT_t[h // 2][r, qs],
                        start=True,
                        stop=True,
                    )
                return sg

            def emit_exp_pv(step, sg):
                h, qb, gi = step
                g0, g1 = GROUPS[gi]
                n = g1 - g0
                pg = pt.tile([P, 3, QB], BF16, name="pg", tag="pt")
                nc.scalar.activation(pg[:, 0:n, :], sg[:, 0:n, :], EXP, scale=0.125)
                return pg

            def emit_pv(step, pg):
                h, qb, gi = step
                g0, g1 = GROUPS[gi]
                for pos in range(g0, g1):
                    nc.tensor.matmul(
                        pvs[(h, qb)],
                        vts[h][:, pos, :],
                        pg[:, pos - g0, :],
                        start=(pos == 0),
                        stop=(pos == NKC - 1),
                    )
                if g1 == NKC:
                    qs = slice(qb * QB, (qb + 1) * QB)
                    nc.vector.tensor_copy(araws[h][:, qs], pvs[(h, qb)])
                    if qb == TOK // QB - 1:
                        finish_head(h)

            pend = [None]

            def flush_norm():
                if pend[0] is None:
                    return
                h, a_raw, bc = pend[0]
                pend[0] = None
                rr = slice(64 * (h % 2), 64 * (h % 2) + 64)
                nc.vector.tensor_mul(lts[h // 2][rr, :], a_raw[0:64, :], bc)

            def finish_head(h):
                with scope(f"norm_h{h}"):
                    a_raw = araws.pop(h)
                    nc.sync.dma_start(out=den_d[h : h + 1, :], in_=a_raw[64:65, :])
                    dsq = sm.tile([64, 16], F32, name="dsq", tag="smd")
                    nc.sync.dma_start(
                        out=dsq,
                        in_=bass.AP(
                            tensor=den_d.tensor,
                            offset=h * TOK,
                            ap=[[16, 64], [1, 16]],
                        ),
                    )
                    rsq = sm.tile([64, 16], F32, name="rsq", tag="smr")
                    nc.vector.reciprocal(rsq, dsq)
                    nc.sync.dma_start(
                        out=bass.AP(
                            tensor=recip_d.tensor,
                            offset=h * TOK,
                            ap=[[16, 64], [1, 16]],
                        ),
                        in_=rsq,
                    )
                    bc = at.tile([64, TOK], F32, name="bc", tag="at2")
                    nc.sync.dma_start(
                        out=bc,
                        in_=bass.AP(
                            tensor=recip_d.tensor,
                            offset=h * TOK,
                            ap=[[0, 64], [1, TOK]],
                        ),
                    )
                    # defer the normalize multiply one head so the DVE never
                    # stalls on the DRAM-bounce latency of the reciprocal
                    flush_norm()
                    pend[0] = (h, a_raw, bc)

            with scope("attn"):
                sg_cur = emit_scores(steps[0])
                for i, step in enumerate(steps):
                    pg = emit_exp_pv(step, sg_cur)
                    if i + 1 < len(steps):
                        sg_cur = emit_scores(steps[i + 1])
                    emit_pv(step, pg)

            flush_norm()
            ps2.close()

            # ---------------- Phase 3: output projection (local) -------------
            with scope("wo"):
                ev3 = ph2.enter_context(tc.tile_pool(name="ev3", bufs=3))
                ps3p = ph2.enter_context(tc.tile_pool(name="ps3p", bufs=3, space="PSUM"))
                for t_i in range(CD):
                    ps3 = ps3p.tile([P, D], F32, name="ps3", tag="ps3")
                    for sc in range(CD):
                        for hh in range(2):
                            nc.tensor.matmul(
                                ps3[:, hh * QB : (hh + 1) * QB],
                                lts[sc][:, t_i * P : (t_i + 1) * P],
                                wo_t[sc][:, hh * QB : (hh + 1) * QB],
                                start=(sc == 0),
                                stop=(sc == CD - 1),
                            )
                    ob = ev3.tile([P, D], F32, name="ob", tag="ev3")
                    nc.vector.tensor_copy(ob, ps3)
                    nc.sync.dma_start(out=out[t_i * P : (t_i + 1) * P, :], in_=ob)

        persist.close()

    return _finish(nc)


def _get_nc(scopes=False):
    key = ("nc", scopes)
    if key not in _CACHE:
        _CACHE[key] = build_nc(scopes)
    return _CACHE[key]


def make_in_maps(query, key, value, wq, wk, wv, wo):
    qf = np.asarray(query, np.float32).reshape(B * S, D)
    kf = np.asarray(key, np.float32).reshape(B * S, D)
    vf = np.asarray(value, np.float32).reshape(B * S, D)
    wqkvT = np.ascontiguousarray(
        np.concatenate([np.asarray(wq), np.asarray(wk), np.asarray(wv)], 0).T
    ).astype(ml_dtypes.bfloat16)
    woT_h = np.ascontiguousarray(np.asarray(wo).T).astype(ml_dtypes.bfloat16)
    in_maps = []
    for c in range(N_CORES):
        sl = slice(c * TOK, (c + 1) * TOK)
        in_maps.append(
            {
                "xqT": np.ascontiguousarray(qf[sl].T).astype(ml_dtypes.bfloat16),
                "xkT": np.ascontiguousarray(kf[sl].T).astype(ml_dtypes.bfloat16),
                "xvT": np.ascontiguousarray(vf[sl].T).astype(ml_dtypes.bfloat16),
                "wqkvT": wqkvT,
                "woT": woT_h,
            }
        )
    return in_maps


def assemble(results):
    blocks = [results[c]["out"] for c in range(N_CORES)]
    return np.concatenate(blocks, 0).reshape(B, S, D).astype(np.float32)


def kernel(query, key, value, mask, wq, wk, wv, wo):
    # mask is all-False in this problem: softmax without masking.
    nc = _get_nc()
    in_maps = make_in_maps(query, key, value, wq, wk, wv, wo)
    from concourse.bass_utils import run_bass_kernel_spmd

    res = run_bass_kernel_spmd(nc, in_maps, list(range(N_CORES)))
    return assemble(res.results)

